# revision 3
# baseline (speedup 1.0000x reference)
"""Trainium2 Bass kernel for nn_MHA_65429531787938 — v3 merged pipeline.

Same math as v2 (batch-softmax -> sigmoid trick, head-parallel over 8 cores,
fp16 operands / fp32 psum, host-side x^T fp16 staging, fp16 0.25-scaled
output partials).

v3 restructures the schedule into ONE loop so the ACT sigmoid stream (the
64us/core critical resource) starts ~10us into the kernel instead of after
all projections:
  - K/V projections for chunk-pair j are interleaved into q-chunk 0's
    tp-slots just before the score matmuls that need them.
  - Q projections for q-chunk qc are deferred into q-chunk qc-1.
  - colsum(V1) correction comes from a GPSIMD pool-avg over vt_sb (free
    engine) and is applied as a per-partition scalar bias in the b=1 vals
    copies (tensor_scalar_sub) — no rank-1 PE matmuls.
  - psum->sbuf copies balanced: K/Q(0) copies on ACT, V/vals/Q(>=2)/out
    copies on DVE; tail out blocks split ACT/DVE.
"""

import numpy as np

import concourse.bacc as bacc
import concourse.mybir as mybir
import concourse.tile as tile
from concourse import bass_utils
from concourse.masks import make_identity

B, S, D, H = 2, 2048, 1024, 16
HD = 64
SCALE = float(D) ** 0.5
NCORES = 8
HPC = H // NCORES            # heads per core = 2
MS = HPC * HD                # per-core slice width = 128
P = 128
DT16 = mybir.dt.float16
F32 = mybir.dt.float32
OSCALE = 0.25                # fp16 partial-output scale (host multiplies by 4)
NQC = S // 512               # q-chunks = 4
NTP = S // P // 2            # k-tile pairs per q-chunk = 8


def build():
    nc = bacc.Bacc("TRN2", target_bir_lowering=False, debug=False)

    xt_d = nc.dram_tensor("xt", [B, D, S], DT16, kind="ExternalInput").ap()
    wq_d = nc.dram_tensor("wq", [D, MS], DT16, kind="ExternalInput").ap()
    wk_d = nc.dram_tensor("wk", [D, MS], DT16, kind="ExternalInput").ap()
    wv_d = nc.dram_tensor("wv", [D, MS], DT16, kind="ExternalInput").ap()
    wo_d = nc.dram_tensor("wo", [MS, D], DT16, kind="ExternalInput").ap()
    out_d = nc.dram_tensor("out", [B, S, D], DT16, kind="ExternalOutput").ap()

    with tile.TileContext(nc) as tc:
        with tc.tile_pool(name="persist", bufs=1) as pp:
            ident16 = pp.tile([P, P], DT16, name="ident16")
            make_identity(nc, ident16[:])

            w_sb = {}
            for name, dram in (("wq", wq_d), ("wk", wk_d), ("wv", wv_d)):
                t = pp.tile([P, D // P, MS], DT16, name=f"{name}_sb")
                nc.gpsimd.dma_start(t[:], dram.rearrange("(t p) m -> p t m", p=P))
                w_sb[name] = t
            wo_sb = pp.tile([P, 2, 512], DT16)
            nc.gpsimd.dma_start(wo_sb[:], wo_d.rearrange("p (c n) -> p c n", c=2))

            qsb = pp.tile([P, HPC, S], DT16)     # [(b,hd), head, qpos], b1 negated
            ksb = pp.tile([P, HPC, S], DT16)     # [(b,hd), head, kpos]
            vt_sb = pp.tile([P, B, S], DT16)     # [(h,hd), batch, kpos], b1 negated
            v_sb = pp.tile([P, S // P, HPC, B, HD], DT16)  # [k, ktile, h, b, hd]
            vals_sb = pp.tile([P, B, S], DT16)   # [(h,hd), batch, qpos]
            c1avg = pp.tile([P, 1], F32)         # avg over kpos of -V1 per (h,d)
            c1col = pp.tile([P, 1], F32)         # -colsum(V1) per (h,d) partition

            with tc.tile_pool(name="p1xt", bufs=8) as p1xt, \
                 tc.tile_pool(name="p2a", bufs=8) as p2a, \
                 tc.tile_pool(name="p3o", bufs=3) as p3o, \
                 tc.tile_pool(name="psA", bufs=2, space="PSUM") as psA, \
                 tc.tile_pool(name="ps2d", bufs=2, space="PSUM") as ps2d, \
                 tc.tile_pool(name="ps2av", bufs=1, space="PSUM") as ps2av:

                xts = {}

                def load_x(b, j, split=False):
                    xt = p1xt.tile([P, D // P, 512], DT16, tag="xt")
                    src = xt_d[b, :, j * 512:(j + 1) * 512].rearrange(
                        "(t p) s -> p t s", p=P)
                    if split:
                        nc.sync.dma_start(xt[:, :4, :], src[:, :4, :])
                        nc.gpsimd.dma_start(xt[:, 4:, :], src[:, 4:, :])
                    else:
                        eng = nc.sync if (2 * j + b) % 2 == 0 else nc.gpsimd
                        eng.dma_start(xt[:], src)
                    xts[(b, j)] = xt

                def proj_qk(name, b, j, dest, neg, act_copy):
                    ps = psA.tile([P, 512], F32, tag="p", name="psA")
                    xt = xts[(b, j)]
                    for t in range(D // P):
                        nc.tensor.matmul(
                            ps[:], w_sb[name][:, t, :], xt[:, t, :],
                            start=(t == 0), stop=(t == D // P - 1),
                        )
                    for h in range(HPC):
                        dst = dest[b * HD:(b + 1) * HD, h, j * 512:(j + 1) * 512]
                        src = ps[h * HD:(h + 1) * HD, :]
                        sc = -1.0 if (neg and b == 1) else 1.0
                        if act_copy:
                            nc.scalar.mul(dst, src, sc)
                        else:
                            nc.vector.tensor_scalar_mul(dst, src, sc)

                def proj_v(b, j):
                    ps = psA.tile([P, 512], F32, tag="p", name="psA")
                    xt = xts[(b, j)]
                    for t in range(D // P):
                        nc.tensor.matmul(
                            ps[:], w_sb["wv"][:, t, :], xt[:, t, :],
                            start=(t == 0), stop=(t == D // P - 1),
                        )
                    nc.vector.tensor_scalar_mul(
                        vt_sb[:, b, j * 512:(j + 1) * 512], ps[:],
                        -1.0 if b == 1 else 1.0,
                    )
                    pvt = psA.tile([P, 4, P], DT16, tag="p", name="psA")
                    for blk in range(4):
                        t = j * 4 + blk
                        nc.tensor.transpose(
                            pvt[:, blk, :], vt_sb[:, b, t * P:(t + 1) * P],
                            ident16[:],
                        )
                    nc.vector.tensor_copy(
                        v_sb[:, j * 4:(j + 1) * 4, :, b, :],
                        pvt[:].rearrange("p t (h d) -> p t h d", h=HPC),
                    )

                def emit_c1():
                    # vt_sb[:, 1, :] holds -V1^T: free-dim sum -> -colsum(V1)
                    nc.vector.reduce_sum(c1col[:], vt_sb[:, 1, :],
                                         axis=mybir.AxisListType.X)

                def emit_out_block(b, si, tail=False):
                    ot = p3o.tile([P, D], DT16, tag="ot", name="ot")
                    for nch in range(2):
                        po = psA.tile([P, 512], F32, tag="p", name="psA")
                        nc.tensor.matmul(
                            po[:],
                            vals_sb[:, b, si * P:(si + 1) * P],
                            wo_sb[:, nch, :],
                            start=True, stop=True,
                        )
                        if tail and nch == 1:
                            nc.scalar.mul(ot[:, nch * 512:(nch + 1) * 512],
                                          po[:], OSCALE)
                        else:
                            nc.vector.tensor_scalar_mul(
                                ot[:, nch * 512:(nch + 1) * 512], po[:], OSCALE
                            )
                    ring = nc.scalar if (tail and si % 2 == 0) else nc.sync
                    ring.dma_start(out_d[b, si * P:(si + 1) * P, :], ot[:])

                # ---- filler schedule: work injected at (qc, tp) boundaries ----
                fill = {}

                def add_fill(qc, tp, fn, *a, **kw):
                    fill.setdefault((qc, tp), []).append((fn, a, kw))

                # qc0: K/V stages j=1..3 + x loads + Q(1) + c1
                add_fill(0, 0, proj_qk, "wk", 0, 1, ksb, False, True)
                add_fill(0, 0, proj_v, 0, 1)
                add_fill(0, 1, proj_qk, "wk", 1, 1, ksb, False, True)
                add_fill(0, 1, proj_v, 1, 1)
                add_fill(0, 2, proj_qk, "wk", 0, 2, ksb, False, True)
                add_fill(0, 2, proj_v, 0, 2)
                add_fill(0, 3, proj_qk, "wk", 1, 2, ksb, False, True)
                add_fill(0, 3, proj_v, 1, 2)
                add_fill(0, 4, proj_qk, "wk", 0, 3, ksb, False, True)
                add_fill(0, 4, proj_v, 0, 3)
                add_fill(0, 5, proj_qk, "wk", 1, 3, ksb, False, True)
                add_fill(0, 5, proj_v, 1, 3)
                add_fill(0, 6, emit_c1)
                add_fill(0, 6, proj_qk, "wq", 0, 1, qsb, True, True)
                add_fill(0, 7, proj_qk, "wq", 1, 1, qsb, True, True)
                # qc1/qc2: deferred Q projections (copies on DVE)
                add_fill(1, 0, proj_qk, "wq", 0, 2, qsb, True, False)
                add_fill(1, 1, proj_qk, "wq", 1, 2, qsb, True, False)
                add_fill(2, 0, proj_qk, "wq", 0, 3, qsb, True, False)
                add_fill(2, 1, proj_qk, "wq", 1, 3, qsb, True, False)

                # ---- prologue: stage 0 (both batches) + Q(0) ----
                load_x(0, 0, split=True)
                for j in range(4):
                    for b in range(B):
                        if (b, j) != (0, 0):
                            load_x(b, j)
                proj_qk("wk", 0, 0, ksb, False, True)
                proj_qk("wq", 0, 0, qsb, True, True)
                proj_qk("wk", 1, 0, ksb, False, True)
                proj_qk("wq", 1, 0, qsb, True, True)
                proj_v(0, 0)
                proj_v(1, 0)

                # ---- merged attention + out-proj loop ----
                for qc in range(NQC):
                    pavs = {}
                    for h in range(HPC):
                        pavs[h] = ps2av.tile([P, 512], F32, tag="av",
                                             name=f"pav{h}")
                    prev_at = None
                    for tp in range(NTP):
                        ats = {}
                        for h in range(HPC):
                            pd = ps2d.tile([P, 1024], F32, tag="d", name="pd")
                            for u in range(2):
                                t = tp * 2 + u
                                nc.tensor.matmul(
                                    pd[:, u * 512:(u + 1) * 512],
                                    ksb[:, h, t * P:(t + 1) * P],
                                    qsb[:, h, qc * 512:(qc + 1) * 512],
                                    start=True, stop=True,
                                )
                            at = p2a.tile([P, 1024], DT16, tag="at", name="at")
                            nc.scalar.activation(
                                at[:], pd[:],
                                mybir.ActivationFunctionType.Sigmoid,
                                scale=1.0 / SCALE,
                            )
                            ats[h] = at
                        if prev_at is not None:
                            ptp, pats = prev_at
                            for h in range(HPC):
                                for u in range(2):
                                    t = ptp * 2 + u
                                    nc.tensor.matmul(
                                        pavs[h][:],
                                        v_sb[:, t, h, :, :].rearrange(
                                            "p b d -> p (b d)"),
                                        pats[h][:, u * 512:(u + 1) * 512],
                                        start=(t == 0), stop=False,
                                    )
                        for fn, a, kw in fill.pop((qc, tp), []):
                            fn(*a, **kw)
                        if qc > 0 and tp < 8:
                            b, sq = divmod(tp, 4)
                            emit_out_block(b, (qc - 1) * 4 + sq)
                        prev_at = (tp, ats)
                    ptp, pats = prev_at
                    for h in range(HPC):
                        for u in range(2):
                            t = ptp * 2 + u
                            nc.tensor.matmul(
                                pavs[h][:],
                                v_sb[:, t, h, :, :].rearrange("p b d -> p (b d)"),
                                pats[h][:, u * 512:(u + 1) * 512],
                                start=False, stop=(u == 1),
                            )
                        # vals copies: b0 plain, b1 with colsum(V1) correction
                        nc.vector.tensor_copy(
                            vals_sb[h * HD:(h + 1) * HD, 0,
                                    qc * 512:(qc + 1) * 512],
                            pavs[h][0:HD, :],
                        )
                        nc.vector.tensor_scalar_sub(
                            vals_sb[h * HD:(h + 1) * HD, 1,
                                    qc * 512:(qc + 1) * 512],
                            pavs[h][HD:2 * HD, :],
                            c1col[h * HD:(h + 1) * HD, :],
                        )
                # trailing out-proj blocks for the last q-chunk
                for b in range(B):
                    for sq in range(4):
                        emit_out_block(b, (NQC - 1) * 4 + sq, tail=True)

    nc.compile()
    return nc


_NC = None


def _get_nc():
    global _NC
    if _NC is None:
        _NC = build()
    return _NC


def kernel(x, w_q, w_k, w_v, W_o, _trace=False):
    x = np.asarray(x, dtype=np.float32)
    x16t = np.ascontiguousarray(
        x.transpose(0, 2, 1).astype(np.float16))          # [B, D, S]
    w_q = np.asarray(w_q, dtype=np.float32)
    w_k = np.asarray(w_k, dtype=np.float32)
    w_v = np.asarray(w_v, dtype=np.float32)
    W_o = np.asarray(W_o, dtype=np.float32)

    nc = _get_nc()
    in_maps = []
    for i in range(NCORES):
        cs = slice(i * MS, (i + 1) * MS)
        in_maps.append({
            "xt": x16t,
            "wq": np.ascontiguousarray(w_q[:, cs].astype(np.float16)),
            "wk": np.ascontiguousarray(w_k[:, cs].astype(np.float16)),
            "wv": np.ascontiguousarray(w_v[:, cs].astype(np.float16)),
            "wo": np.ascontiguousarray(W_o[cs, :].astype(np.float16)),
        })
    try:
        res = bass_utils.run_bass_kernel_spmd(
            nc, in_maps, core_ids=list(range(NCORES)), trace=_trace
        )
    except Exception:
        res = bass_utils.run_bass_kernel_spmd(
            nc, in_maps, core_ids=list(range(NCORES)), trace=_trace
        )
    out = res.results[0]["out"].astype(np.float32)
    for i in range(1, NCORES):
        out += res.results[i]["out"].astype(np.float32)
    out *= 1.0 / OSCALE
    if _trace:
        return out, res
    return out
